# revision 55
# baseline (speedup 1.0000x reference)
"""Channel-attention Trainium2 Bass kernel, Gram-collapsed + fp8 DoubleRow.

Key identity: this is CHANNEL attention (the softmax mixes the 64 channels
of each head; every pixel is treated identically), so the whole module
collapses to a per-batch 256x256 effective channel-mixing matrix:

    G     = x^T x                     # [256,256] Gram, contracts d=4096
    sim_h = wq_h^T G wk_h             # [64,64] per head  (== (x wq)^T (x wk))
    attn_h = softmax(sim_h)           # denominator folded into wo rows
    M_h   = attn_h^T wo_h             # [64,256]
    W     = wv @ concat_h(M_h)        # [256,256] effective weight
    y     = x @ W (+ b_out)

Only G and y touch the [4096, 256] data; both run as fp8e4m3 DoubleRow
matmuls (0.5 cyc/row, 2 K-tiles per instruction) on hi+lo residual pairs:
a @ b ~= ah@bh + al@bh + ah@bl, where the lo tensors carry the fp8
quantization residual of the hi ones. That keeps fp16-grade accuracy
(end-to-end rel-l2 ~3.4e-3 vs the fp64 oracle) at fp8 speed and the same
DMA bytes as fp16. Scales are powers of two: x_dc*2 (so Gsb=4G stays
under fp16 max), xT*16, W*256; the q-scale/8, G/4 land in w_q host-side
and the 4096x on y divides out on the host.

Softmax denominators never touch e: attn = e/s is realized by scaling
wo's rows by r = 1/s (per-partition tensor_scalar) before the M matmul,
since M's contraction index (attn row i) is exactly wo's row index.

Distribution: data-parallel over batch - 8 cores x 2 batches each, weights
replicated, no collectives. Per-core DMA is the roofline (~11MB at
360GB/s ~= 32us): x twice (d-major for G, c-major for y, 2MB/batch each),
y out fp8e3 (1MB/batch), weights 1MB. DMA instruction count stays small
(~30/core, ~625ns serialized issue each) and every transfer keeps >=1KB
contiguous per-partition runs for full bandwidth. PE work is ~32k
column-cycles/batch (~27us/core), mostly hidden under the DMA stream.
PSUM drains go on DVE/Act only (GPSIMD cannot touch PSUM on this HW).
"""

import numpy as np
import ml_dtypes

import concourse.bass as bass
import concourse.mybir as mybir
from concourse.bass_utils import run_bass_kernel_spmd
from concourse.tile import TileContext

DR = mybir.MatmulPerfMode.DoubleRow


def _split_multi_waits(nc, limit=1):
    """Post-pass: the walrus build in this container rejects instructions
    carrying more than `limit` sync-waits ("Too many sync wait commands" in
    setupSyncWait). Tile attaches up to 3. Hoist the extras onto same-engine
    NoOp instructions inserted immediately before the owner — the engine
    sequencer executes them in order, so the ordering semantics are
    identical."""
    drain_engines = [
        mybir.EngineType.PE,
        mybir.EngineType.DVE,
        mybir.EngineType.Activation,
        mybir.EngineType.Pool,
        mybir.EngineType.SP,
    ]
    n_split = 0
    for f in nc.m.functions:
        for blk in f.blocks:
            il = blk.instructions
            i = 0
            while i < len(il):
                inst = il[i]
                si = inst.sync_info
                waits = list(si.on_wait) if si is not None else []
                if len(waits) > limit:
                    si.on_wait = waits[:limit]
                    is_drain = type(inst).__name__ == "InstDrain"
                    for k, w in enumerate(waits[limit:]):
                        nop = mybir.InstNoOp(
                            name=f"I-waitsplit-{n_split}", ins=[], outs=[]
                        )
                        n_split += 1
                        nop.engine = (
                            drain_engines[k % len(drain_engines)]
                            if is_drain else inst.engine
                        )
                        nop.sync_info = mybir.SyncInfo(on_wait=[w], on_update=[])
                        il.insert(i, nop)
                        i += 1
                i += 1
    return nc


N_CORES = 8
BATCH = 16
BPC = BATCH // N_CORES  # batches per core
D = 4096  # spatial (64*64)
C = 256   # channels
HID = 512
HEADS = 8

F32 = mybir.dt.float32
F16 = mybir.dt.float16
F8 = mybir.dt.float8e4
F8E3 = mybir.dt.float8e3
E4NP = ml_dtypes.float8_e4m3
E3NP = ml_dtypes.float8_e3m4

# offsets into the packed weight tile w_all [128, 4096] (fp16)
WK_OFF = 0          # wk  [128, 2, 512]
WQ_OFF = 1024       # wq' [128, 2, 512]  (q-scale/8 and Gram-scale/4 folded)
WVT_OFF = 2048      # wvT [128, 4, 256]
WO_OFF = 3072       # wo  [128, 4, 256]

_CACHE = {}


def _build():
    nc = bass.Bass()
    # x twice: d-major (partition = d%128) for G, c-major for Y; each as an
    # fp8 hi/lo residual pair (same bytes as fp16)
    xdh_d = nc.declare_dram_parameter("xdc_hi", [BPC, 128, 32 * C], F8, isOutput=False)
    xdl_d = nc.declare_dram_parameter("xdc_lo", [BPC, 128, 32 * C], F8, isOutput=False)
    xth_d = nc.declare_dram_parameter("xT_hi", [BPC, 128, 2 * D], F8, isOutput=False)
    xtl_d = nc.declare_dram_parameter("xT_lo", [BPC, 128, 2 * D], F8, isOutput=False)
    w_d = nc.declare_dram_parameter("w_all", [128, 4096], F16, isOutput=False)
    # y leaves as fp8e3m4 (4 mantissa bits) at scale 2: ~1.2% quantization,
    # well inside the 2e-2 gate, and it halves the y DMA bytes
    y_d = nc.declare_dram_parameter("y", [BPC, 128, 2 * D], F8E3, isOutput=True)

    with TileContext(nc) as tc:
        with (
            tc.tile_pool(name="consts", bufs=1) as consts,
            tc.tile_pool(name="xdc", bufs=2) as xdc_pool,
            tc.tile_pool(name="xt", bufs=2) as xt_pool,
            tc.tile_pool(name="small", bufs=2) as small_pool,
            tc.tile_pool(name="e64", bufs=2) as e_pool,
            tc.tile_pool(name="stat", bufs=6) as stat_pool,
            tc.tile_pool(name="ysb", bufs=2) as y_pool,
            tc.tile_pool(name="mm", bufs=6, space="PSUM") as mm_pool,
            tc.tile_pool(name="simp", bufs=2, space="PSUM") as sim_pool,
        ):
            w_all = consts.tile([128, 4096], F16, name="w_all")

            # PE p-state warmup: ~5us of dummy matmuls on a zeroed tile so
            # G0's real matmuls start at the full 2.4GHz clock instead of
            # spending their first 3us at the 1.2GHz ramp rate
            warm = consts.tile([128, HID], F16, name="warm")
            nc.gpsimd.memset(warm, 0.0)
            for wi in range(8):
                wps = sim_pool.tile([128, HID], F32, name="warmp", tag="simp")
                nc.tensor.matmul(
                    wps, lhsT=warm[:, 0:128], rhs=warm, start=True, stop=True
                )

            def wk(j):  # [128, 512] c-chunk j
                return w_all[:, WK_OFF + j * HID:WK_OFF + (j + 1) * HID]

            def wq(j, h):  # [128, 64] c-chunk j, head h
                lo = WQ_OFF + j * HID + h * 64
                return w_all[:, lo:lo + 64]

            def wvt(t, m):  # [128, 128]: hid-chunk t, c-half m
                lo = WVT_OFF + t * C + m * 128
                return w_all[:, lo:lo + 128]

            def wo(p):  # [128, 256] rows of head pair p
                lo = WO_OFF + p * C
                return w_all[:, lo:lo + C]

            # ---- x/w tiles + DMA stream (order = issue order) ----
            xdh, xdl, xth, xtl = [], [], [], []
            for b in range(BPC):
                xdh.append(xdc_pool.tile([128, 32, C], F8, name=f"xdh{b}", tag="xdh"))
                xdl.append(xdc_pool.tile([128, 32, C], F8, name=f"xdl{b}", tag="xdl"))
                xth.append(xt_pool.tile([128, 2, D], F8, name=f"xth{b}", tag="xth"))
                xtl.append(xt_pool.tile([128, 2, D], F8, name=f"xtl{b}", tag="xtl"))

            def dma_xdc(b, half):
                for q in (2 * half, 2 * half + 1):
                    ks = slice(q * 8, (q + 1) * 8)
                    el = slice(q * 8 * C, (q + 1) * 8 * C)
                    nc.sync.dma_start(out=xdh[b][:, ks, :], in_=xdh_d[b, :, el])
                    nc.sync.dma_start(out=xdl[b][:, ks, :], in_=xdl_d[b, :, el])

            dma_xdc(0, 0)
            dma_xdc(0, 1)
            nc.sync.dma_start(out=w_all[:, 0:2048], in_=w_d[:, 0:2048])
            dma_xdc(1, 0)
            dma_xdc(1, 1)
            nc.sync.dma_start(out=w_all[:, 2048:4096], in_=w_d[:, 2048:4096])
            def dma_xt(tile, dram, b):
                # (c-chunk, d-half) quarters: the d-half-0 pieces land first
                # so the Y phase's first d5 groups unblock ~1.5us earlier
                for half in range(2):
                    for t in range(2):
                        lo = half * 2048
                        nc.sync.dma_start(
                            out=tile[:, t, lo:lo + 2048],
                            in_=dram[b, :, t * D + lo:t * D + lo + 2048],
                        )

            for b in range(BPC):
                dma_xt(xth[b], xth_d, b)
                dma_xt(xtl[b], xtl_d, b)

            def emit_g(b):
                """G = (xh+xl)^T(xh+xl) (3-term) via fp8 DoubleRow over
                d-chunk pairs; PSUM = 4G -> Gsb fp16. Emitted half-by-half
                so the first half's matmuls start under the second's DMA."""
                gps = [
                    mm_pool.tile([128, HID], F32, name=f"gps{m}", tag="mm")
                    for m in range(2)
                ]
                for quarter in range(4):
                    for ab, (lt, rt) in enumerate(
                        ((xdh, xdh), (xdl, xdh), (xdh, xdl))
                    ):
                        for pp in range(4):
                            p2 = quarter * 8 + pp * 2
                            ks = slice(p2, p2 + 2)
                            for m in range(2):
                                nc.tensor.matmul(
                                    gps[m][:, 0:C],
                                    lhsT=lt[b][:, ks, m * 128:(m + 1) * 128],
                                    rhs=rt[b][:, ks, :],
                                    start=(quarter == 0 and ab == 0 and pp == 0),
                                    stop=(quarter == 3 and ab == 2 and pp == 3),
                                    perf_mode=DR,
                                )
                g_sb = small_pool.tile([128, 2, C], F16, name="gsb", tag="gsb")
                # both copies are on the t2 critical path: use two engines
                nc.vector.tensor_copy(g_sb[:, 0, :], gps[0][:, 0:C])
                nc.scalar.copy(g_sb[:, 1, :], gps[1][:, 0:C])
                return g_sb

            def emit_t2_sim(b, g_sb):
                """t2 = G @ wk -> fp16; sim_h = wq_h^T t2_h -> PSUM.

                G is exactly symmetric (both halves accumulate the same
                products in the same order), so Gsb tile j doubles as the
                [c2-chunk j, c1] stationary operand."""
                t2_sb = small_pool.tile([128, 2, HID], F16, name="t2sb", tag="t2")
                for m in range(2):
                    t2p = mm_pool.tile([128, HID], F32, name="t2p", tag="mm")
                    for j in range(2):
                        nc.tensor.matmul(
                            t2p,
                            lhsT=g_sb[:, j, m * 128:(m + 1) * 128],
                            rhs=wk(j),
                            start=(j == 0),
                            stop=(j == 1),
                        )
                    if m == 0:
                        nc.vector.tensor_copy(t2_sb[:, m, :], t2p)
                    else:
                        nc.scalar.copy(t2_sb[:, m, :], t2p)
                # sim packing: head h=(2p+par) -> rows par*64:+64, cols
                # p*64:+64 of sim_all [128, 256]
                sim_all = sim_pool.tile([128, HID], F32, name="sim", tag="simp")
                nc.vector.memset(sim_all[:, 0:C], 0.0)
                for h in range(HEADS):
                    par, p = h % 2, h // 2
                    rows = slice(par * 64, par * 64 + 64)
                    for j in range(2):
                        nc.tensor.matmul(
                            sim_all[rows, p * 64:(p + 1) * 64],
                            lhsT=wq(j, h),
                            rhs=t2_sb[:, j, h * 64:(h + 1) * 64],
                            start=False,
                            stop=(j == 1),
                            skip_group_check=True,
                        )
                return sim_all

            def emit_softmax_stats(b, sim_all):
                """exp (grouped, max-subtracted) -> e64 fp16; 1/s folded
                into wo rows (per-partition scale, since M's contraction
                index is wo's row index). DVE/Act/Pool only - emitted right
                after the sim matmuls so these never queue behind the
                y-phase PSUM drains on the same engines."""
                m_t = stat_pool.tile([128, 4], F32, name="m_t", tag="stat")
                s_t = stat_pool.tile([128, 4], F32, name="s_t", tag="stat")
                r_t = stat_pool.tile([128, 4], F32, name="r_t", tag="stat")
                e64 = e_pool.tile([128, 4, 64], F16, name="e64", tag="e64")
                # neg-max per (partition, head-block): [64,4,64] -> [64,4]
                for par in range(2):
                    rows = slice(par * 64, par * 64 + 64)
                    nc.vector.reduce_max(
                        out=m_t[rows, 0:4],
                        in_=sim_all[rows, 0:C].rearrange("p (g j) -> p g j", g=4),
                        axis=mybir.AxisListType.X,
                        negate=True,
                    )
                for p in range(4):
                    nc.scalar.activation(
                        out=e64[:, p, :],
                        in_=sim_all[:, p * 64:(p + 1) * 64],
                        func=mybir.ActivationFunctionType.Exp,
                        bias=m_t[:, p:p + 1],
                        scale=1.0,
                        accum_out=s_t[:, p:p + 1],
                    )
                nc.vector.reciprocal(r_t, s_t)
                wops = []
                for p in range(4):
                    wop = stat_pool.tile([128, C], F16, name=f"wop{p}", tag="wop")
                    nc.vector.tensor_scalar_mul(wop, wo(p), r_t[:, p:p + 1])
                    wops.append(wop)
                return e64, wops

            def emit_m_weff(b, e64, wops):
                """M via K=64 matmuls; W = wv @ M -> fp8 hi/lo at scale 256."""
                m_sb = small_pool.tile([128, 4, C], F16, name="msb", tag="msb")
                for p in range(4):
                    wop = wops[p]
                    mp = mm_pool.tile([128, HID], F32, name="mp", tag="mm")
                    for par in range(2):
                        rows = slice(par * 64, par * 64 + 64)
                        nc.tensor.matmul(
                            mp[rows, 0:C],
                            lhsT=e64[rows, p, :],
                            rhs=wop[rows, :],
                            start=True,
                            stop=True,
                        )
                    if p % 2 == 0:
                        nc.scalar.copy(m_sb[:, p, :], mp[:, 0:C])
                    else:
                        nc.vector.tensor_copy(m_sb[:, p, :], mp[:, 0:C])
                w_hi = small_pool.tile([128, 2, C], F8, name="whi", tag="whi")
                w_lo = small_pool.tile([128, 2, C], F8, name="wlo", tag="wlo")
                for m in range(2):
                    wp = mm_pool.tile([128, HID], F32, name="wp", tag="mm")
                    for t in range(4):
                        nc.tensor.matmul(
                            wp[:, 0:C],
                            lhsT=wvt(t, m),
                            rhs=m_sb[:, t, :],
                            start=(t == 0),
                            stop=(t == 3),
                        )
                    # W * 256 as fp8 hi + residual lo, straight from PSUM:
                    # hi = (wp * 256) cast on Act; lo = (wp * 256) - hi on DVE
                    nc.scalar.mul(w_hi[:, m, :], wp[:, 0:C], 256.0)
                    nc.vector.scalar_tensor_tensor(
                        out=w_lo[:, m, :],
                        in0=wp[:, 0:C],
                        scalar=256.0,
                        in1=w_hi[:, m, :],
                        op0=mybir.AluOpType.mult,
                        op1=mybir.AluOpType.subtract,
                    )
                return w_hi, w_lo

            def emit_y(b, w_hi, w_lo, yt_sb, d4s, look=1):
                """yT = W^T x^T: fp8 DoubleRow, W halves stationary, xT
                moving; PSUM = 4096*y -> fp8e3 at 2y (host divides).

                The xtl-dependent third term of each group is emitted one
                group late so the first two terms (which only need xth)
                keep PE busy while the xT_lo DMA is still in flight."""
                groups = [(d4, m, dd) for d4 in d4s for m in range(2)
                          for dd in range(2)]
                yps = {}

                def cols_of(g):
                    d4, m, dd = g
                    return m, slice((d4 * 2 + dd) * 512, (d4 * 2 + dd + 1) * 512)

                def emit_t12(g):
                    m, cols = cols_of(g)
                    yp = mm_pool.tile([128, HID], F32, name="yp", tag="mm")
                    yps[g] = yp
                    for ti, lt in enumerate((w_hi, w_lo)):
                        nc.tensor.matmul(
                            yp,
                            lhsT=lt[:, :, m * 128:(m + 1) * 128],
                            rhs=xth[b][:, :, cols],
                            start=(ti == 0),
                            stop=False,
                            perf_mode=DR,
                        )

                def emit_t3(g):
                    m, cols = cols_of(g)
                    nc.tensor.matmul(
                        yps[g],
                        lhsT=w_hi[:, :, m * 128:(m + 1) * 128],
                        rhs=xtl[b][:, :, cols],
                        start=False,
                        stop=True,
                        perf_mode=DR,
                    )
                    # 2*y = PSUM * 2^-11, cast to fp8e3; rotate engines
                    d4, m_, dd = g
                    eng = (d4 * 4 + m_ * 2 + dd) % 2
                    if eng == 0:
                        nc.scalar.mul(yt_sb[:, m_, cols], yps[g], 2.0 ** -11)
                    else:
                        nc.vector.tensor_scalar_mul(
                            yt_sb[:, m_, cols], yps[g], 2.0 ** -11
                        )
                    if dd == 1:
                        lo = d4 * 1024
                        nc.sync.dma_start(
                            out=y_d[b, :, m_ * D + lo:m_ * D + lo + 1024],
                            in_=yt_sb[:, m_, lo:lo + 1024],
                        )

                for i in range(len(groups) + look):
                    if i < len(groups):
                        emit_t12(groups[i])
                    if i >= look:
                        emit_t3(groups[i - look])

            # ---- schedule: G0 t2/sim0 | G1 (PE busy during softmax0) |
            #      M0 W0 | t2/sim1 Y0... M1 W1 (under Y0 tail) ...Y0 Y1 ----
            yts = [
                y_pool.tile([128, 2, D], F8E3, name=f"ysb{b}", tag="ysb")
                for b in range(BPC)
            ]
            g0 = emit_g(0)
            s0 = emit_t2_sim(0, g0)
            st0 = emit_softmax_stats(0, s0)
            g1 = emit_g(1)
            s1 = emit_t2_sim(1, g1)
            wh0, wl0 = emit_m_weff(0, *st0)
            st1 = emit_softmax_stats(1, s1)
            emit_y(0, wh0, wl0, yts[0], range(0, 2), look=1)
            wh1, wl1 = emit_m_weff(1, *st1)
            emit_y(0, wh0, wl0, yts[0], range(2, 4), look=1)
            emit_y(1, wh1, wl1, yts[1], range(0, 4), look=2)
    return _split_multi_waits(nc)


def _get_nc():
    if "nc" not in _CACHE:
        _CACHE["nc"] = _build()
    return _CACHE["nc"]


def _hilo(x, scale):
    """fp8e4m3 hi + residual lo of x*scale (f32 in, ml_dtypes out)."""
    xs = (x * scale).astype(np.float32)
    hi = xs.astype(E4NP)
    lo = (xs - hi.astype(np.float32)).astype(E4NP)
    return hi, lo


def kernel(x, w_qkv, w_out, b_out, **kw):
    x = np.asarray(x, dtype=np.float32)
    w_qkv = np.asarray(w_qkv, dtype=np.float32)
    w_out = np.asarray(w_out, dtype=np.float32)
    b_out = np.asarray(b_out, dtype=np.float32)

    # fold q-scale/8 and Gram-scale/4 into w_q; pack weights into one
    # [128, 4096] fp16 tile: [wk | wq' | wvT | wo], each c/hid-chunked so
    # partition p holds row t*128+p of the logical matrix in slot t
    wq = (w_qkv[:, :HID] * (64 ** -0.5) * 0.25).astype(np.float16)
    wk = w_qkv[:, HID:2 * HID].astype(np.float16)
    wvT = np.ascontiguousarray(w_qkv[:, 2 * HID:].T).astype(np.float16)
    wo = w_out.astype(np.float16)
    w_all = np.concatenate([
        wk.reshape(2, 128, HID).transpose(1, 0, 2).reshape(128, 2 * HID),
        wq.reshape(2, 128, HID).transpose(1, 0, 2).reshape(128, 2 * HID),
        wvT.reshape(4, 128, C).transpose(1, 0, 2).reshape(128, 4 * C),
        wo.reshape(4, 128, C).transpose(1, 0, 2).reshape(128, 4 * C),
    ], axis=1)
    w_all = np.ascontiguousarray(w_all)

    x4 = x.reshape(BATCH, D, C)
    in_maps = []
    for core in range(N_CORES):
        xb = x4[core * BPC:(core + 1) * BPC]  # [BPC, D, C] f32
        # d-major: partition p <- row k*128+p, free slot k; scale 2
        x_dc = np.ascontiguousarray(
            xb.reshape(BPC, 32, 128, C).transpose(0, 2, 1, 3)
        ).reshape(BPC, 128, 32 * C)
        xdh, xdl = _hilo(x_dc, 2.0)
        # c-major: partition p <- channel t*128+p, free slot t; scale 16
        x_t = np.ascontiguousarray(
            xb.transpose(0, 2, 1).reshape(BPC, 2, 128, D).transpose(0, 2, 1, 3)
        ).reshape(BPC, 128, 2 * D)
        xth, xtl = _hilo(x_t, 16.0)
        in_maps.append({
            "xdc_hi": xdh, "xdc_lo": xdl, "xT_hi": xth, "xT_lo": xtl,
            "w_all": w_all,
        })

    nc = _get_nc()
    res = run_bass_kernel_spmd(nc, in_maps, core_ids=list(range(N_CORES)), **kw)
    # y arrives as 2*yT in fp8e3 [BPC, 128, 2, D]: channel t*128+p, pixel d
    def as_e3(a):
        a = np.asarray(a)
        return a if a.dtype == E3NP else a.view(E3NP)

    yt = np.stack([as_e3(r["y"]) for r in res.results])
    yt = yt.reshape(BATCH, 128, 2, D).transpose(0, 2, 1, 3).reshape(BATCH, C, D)
    y = yt.transpose(0, 2, 1).astype(np.float32) * 0.5 + b_out
    return y.reshape(BATCH, 64, 64, C)
